# revision 14
# baseline (speedup 1.0000x reference)
"""Trainium2 Bass kernel for a 4-layer compressed model:

    for l in range(4):  x = x @ (base[l] + bitdelta[l] * mask[l])

x: [16, 4096] f32, base/mask: [4, 4096, 4096] f32, bitdelta: [4] f32.

Sharding (8 cores, tensor parallel on weight columns):
  core c owns columns [c*512, (c+1)*512) of every layer's weight.

Key ideas (vs the 160 us CC-collective version):
  * NO on-chip weight reconstruction:
      x @ (base + bd*mask) = (bd*x) @ mask  +  x @ (64*base) / 64
    with both weight streams in fp8e4 (mask +-1 exact; 64*base ~2.4%
    err on a ~2.5%-weight term). 16 MiB HBM traffic per core, streamed
    as 16 x 1 MiB HWDGE DMAs up front, all tiles resident in SBUF.
  * Two concurrent PE column-group streams (bd*x stationary at cols
    0-15, x at cols 32-47; bf16 stationary x fp8 moving), one PSUM
    bank, partitions 0-15 / 32-47.
  * Recombine+transpose via two accumulating "selector" matmuls per
    128-col chunk, all in PE row group 0 (row-group alternation inside
    an accumulation stream crashes the device).
  * The inter-layer activation AllGather does NOT use collective_compute
    (the CC firmware has a ~75 us first-use init + ~10 us per-op launch
    latency).  Instead each core broadcasts its yT shard [128,64] bf16
    SBUF->SBUF to all 8 cores (self included) with single-slot
    remote_dma_broadcast: slot k reaches peer my_tpb^k (relative XOR
    addressing -> identical SPMD program), landing at slot-k offset of
    the receiver's gather buffer.  Cross-die slots (k>=4) carry an
    empirical ^2 route aberration, compensated by passing (0, k^2).
    Each transfer bumps the receiver's remote sem by 2 only after all
    bytes land; consumers wait rs >= 16*(round+1).  The gathered x
    block order is therefore r^k -- compensated by XOR-permuting each
    core's host-prepared weight (and x0) row blocks.
  * A fire-and-forget dummy broadcast round at t~0 warms the SWDGE
    rings/routes (cold first-execution exchanges can mis-deliver).
  * All exchange instructions + the remote-sem waits sit in
    tc.tile_critical() blocks (Tile's scheduling simulator cannot model
    remotely-incremented semaphores).

Output is written transposed ([512,16] f32 per core); the host
concatenates and transposes back.
"""

import numpy as np

import concourse.bass as bass
import concourse.mybir as mybir
import concourse.tile as tile
from concourse import bacc
from concourse.bass_utils import run_bass_kernel_spmd

L = 4
D = 4096
B = 16
NCORES = 8
C = D // NCORES          # 512 columns per core
KT = D // 128            # 32 contraction tiles of 128
GK = 8                   # k-tiles per DMA chunk
NCH = KT // GK           # 4 chunks per layer (1 MiB each: mask+base)
CT = C // 128            # 4 transpose chunks
SCALE = 64.0             # base pre-scale (fp8 denormal avoidance)

F32 = mybir.dt.float32
BF16 = mybir.dt.bfloat16
FP8 = mybir.dt.float8e4
ALU = mybir.AluOpType

_cache = {}


def _xor_exchange(nc, out_tile, src_ap, rs, ls, prep, prep_base):
    """8 single-slot broadcasts: slot k -> peer my_tpb^k, data lands at
    out_tile[:, 64k:64k+64]. Cross-die slots pass (0, k^2) to undo the
    RMTV-balance route aberration. Caller waits rs afterwards."""
    for k in range(NCORES):
        rd = [None] * 8
        rd[k] = (0, k ^ 2 if k >= 4 else k)
        nc.gpsimd.remote_dma_broadcast(
            out_ap=out_tile[:, k * CT * B:(k + 1) * CT * B],
            in_ap=src_ap,
            remote_sem=rs,
            local_sem=ls,
            rdests=rd,
        ).then_inc(prep, 1)
    nc.gpsimd.wait_ge(prep, prep_base + 8)
    nc.gpsimd.trigger_dma(count=8)


def build(bd_vals):
    nc = bacc.Bacc(
        "TRN2",
        target_bir_lowering=False,
        debug=False,
        num_devices=NCORES,
    )

    # x0^T per core, XOR-block-permuted: [p, 64k+16f+b] =
    #   x0T[512*(r^k) + 128f + p, b]
    xT0 = nc.dram_tensor("xT0", [128, NCORES * CT * B], BF16,
                         kind="ExternalInput")
    # selector: cols 0-15 I (mask acc), cols 16-31 I/64 (base acc)
    sel = nc.dram_tensor("sel", [B, 2 * B], BF16, kind="ExternalInput")
    # weight chunks, rows XOR-permuted to match the gathered x layout:
    # [l, g, p, j*C+c] = W_perm[128*(8g+j) + p, c],
    # W_perm rows = [512*(r^k)+i for k in 0..7 for i in 0..511]
    # first half of the free dim per j is mask, second half 64*base.
    w8 = nc.dram_tensor("w8", [L, NCH, 128, 2 * GK * C], FP8,
                        kind="ExternalInput")
    outT = nc.dram_tensor("outT", [C, B], F32, kind="ExternalOutput")

    with tile.TileContext(nc) as tc:
        with (
            tc.tile_pool(name="w", bufs=L * NCH) as wpool,
            tc.tile_pool(name="xp", bufs=2) as xpool,
            tc.tile_pool(name="xg", bufs=L) as xgpool,
            tc.tile_pool(name="sp", bufs=2) as spool,
            tc.tile_pool(name="const", bufs=1) as cpool,
            tc.tile_pool(name="acc", bufs=2, space="PSUM") as psum,
            tc.tile_pool(name="tp", bufs=2, space="PSUM") as tpsum,
            nc.semaphore("rs") as rs,
            nc.semaphore("rsw") as rsw,
            nc.semaphore("ls") as ls,
            nc.semaphore("prep") as prep,
        ):
            sel_sb = cpool.tile([B, 2 * B], BF16, tag="sel")
            nc.scalar.dma_start(sel_sb[:, :], sel[:, :])

            xg = xgpool.tile([128, NCORES * CT * B], BF16, tag="xg")
            nc.scalar.dma_start(xg[:, :], xT0[:, :])

            # whole weight stream up front on the sync HWDGE ring
            wt = []
            for l in range(L):
                for g in range(NCH):
                    t = wpool.tile([128, 2 * GK * C], FP8, tag="w")
                    nc.sync.dma_start(t[:, :], w8[l, g])
                    wt.append(t)

            # dummy warm-up broadcast round: warms SWDGE rings + routes.
            # Fire-and-forget (separate sem, nobody waits).
            scratch = cpool.tile([128, NCORES * CT * B], BF16, tag="scratch")
            with tc.tile_critical():
                _xor_exchange(nc, scratch, xg[:, 0:CT * B], rsw, ls, prep, 0)

            nexch = 0
            for l in range(L):
                last = l == L - 1
                bd = float(bd_vals[l])

                xm = xpool.tile([128, NCORES * CT * B], BF16, tag="xm")
                nc.vector.tensor_scalar_mul(xm[:, :], xg[:, :], bd)

                ps = psum.tile([48, C], F32, tag="ps")
                for k in range(KT):
                    g, j = k // GK, k % GK
                    wti = wt[l * NCH + g]
                    nc.tensor.matmul(
                        ps[0:16, :],
                        xm[:, k * B:(k + 1) * B],
                        wti[:, j * C:(j + 1) * C],
                        start=(k == 0), stop=(k == KT - 1),
                        tile_position=(0, 0), skip_group_check=True,
                    )
                    nc.tensor.matmul(
                        ps[32:48, :],
                        xg[:, k * B:(k + 1) * B],
                        wti[:, GK * C + j * C:GK * C + (j + 1) * C],
                        start=(k == 0), stop=(k == KT - 1),
                        tile_position=(0, 32), skip_group_check=True,
                    )

                # drain both accumulators to partition-0 stacks (ACT can
                # shift partitions: PSUM[32:48] -> SBUF[0:16])
                stka = spool.tile([B, C], BF16, tag="stka")
                stkb = spool.tile([B, C], BF16, tag="stkb")
                nc.scalar.copy(stka[:, :], ps[0:16, :])
                nc.scalar.copy(stkb[:, :], ps[32:48, :])

                # recombine + transpose: yT_chunk = A.T @ I + B.T @ (I/64)
                yt_ps = tpsum.tile([128, CT * B], F32, tag="ytps")
                for cc in range(CT):
                    nc.tensor.matmul(
                        yt_ps[:, cc * B:(cc + 1) * B],
                        stka[:, cc * 128:(cc + 1) * 128],
                        sel_sb[:, 0:B],
                        start=True, stop=False,
                        skip_group_check=True,
                    )
                    nc.tensor.matmul(
                        yt_ps[:, cc * B:(cc + 1) * B],
                        stkb[:, cc * 128:(cc + 1) * 128],
                        sel_sb[:, B:2 * B],
                        start=False, stop=True,
                        skip_group_check=True,
                    )

                if last:
                    ytf = spool.tile([128, CT * B], F32, tag="ytf")
                    nc.scalar.copy(ytf[:, :], yt_ps[:, :])
                    nc.scalar.dma_start(
                        outT[:, :].rearrange("(cc p) b -> p cc b", p=128),
                        ytf[:, :].rearrange("p (cc b) -> p cc b", cc=CT),
                    )
                else:
                    yt_sb = spool.tile([128, CT * B], BF16, tag="ytsb")
                    nc.scalar.copy(yt_sb[:, :], yt_ps[:, :])

                    xg = xgpool.tile([128, NCORES * CT * B], BF16, tag="xg")
                    nexch += 1
                    with tc.tile_critical():
                        _xor_exchange(nc, xg, yt_sb[:, :], rs, ls, prep,
                                      8 * nexch)
                        nc.tensor.wait_ge(rs, 16 * nexch)
                        nc.vector.wait_ge(rs, 16 * nexch)

    nc.compile()
    return nc


def _get_nc(bd_vals):
    key = tuple(float(v) for v in bd_vals)
    if _cache.get("key") != key:
        _cache["nc"] = build(bd_vals)
        _cache["key"] = key
    return _cache["nc"]


def _make_in_maps(x, base, mask, bitdelta):
    import ml_dtypes

    x = np.ascontiguousarray(np.asarray(x, dtype=np.float32))
    base = np.asarray(base, dtype=np.float32)
    mask = np.asarray(mask, dtype=np.float32)

    xT = np.ascontiguousarray(x.T)                   # [D, B] f32

    sel = np.zeros((B, 2 * B), dtype=np.float32)
    sel[:, 0:B] = np.eye(B, dtype=np.float32)
    sel[:, B:2 * B] = np.eye(B, dtype=np.float32) / SCALE
    sel = sel.astype(ml_dtypes.bfloat16)

    mask8 = mask.astype(ml_dtypes.float8_e4m3)
    base8 = (base * SCALE).astype(ml_dtypes.float8_e4m3)

    # x0T blocks [8, CT, 128, B]
    xblk = xT.reshape(NCORES, CT, 128, B)
    in_maps = []
    for r in range(NCORES):
        perm = [r ^ k for k in range(NCORES)]
        # xT0: [p, (k, f, b)] = x0T[512*(r^k) + 128f + p, b]
        xg0 = xblk[perm]                              # [8, CT, 128, B]
        xg0 = np.ascontiguousarray(
            xg0.transpose(2, 0, 1, 3).reshape(128, NCORES * CT * B)
        ).astype(ml_dtypes.bfloat16)

        sl = slice(r * C, (r + 1) * C)
        m = mask8[:, :, sl].reshape(L, NCORES, C, C)[:, perm]   # [L,8,512,C]
        b = base8[:, :, sl].reshape(L, NCORES, C, C)[:, perm]
        # -> W_perm [L, 4096, C] -> k-tiles [L, KT, 128, C]
        m = m.reshape(L, KT, 128, C)
        b = b.reshape(L, KT, 128, C)
        w = np.empty((L, NCH, 128, 2 * GK * C), dtype=ml_dtypes.float8_e4m3)
        for g in range(NCH):
            gs = slice(g * GK, (g + 1) * GK)
            # [L, GK, 128, C] -> [L, 128, GK, C]
            w[:, g, :, :GK * C] = m[:, gs].transpose(0, 2, 1, 3).reshape(
                L, 128, GK * C)
            w[:, g, :, GK * C:] = b[:, gs].transpose(0, 2, 1, 3).reshape(
                L, 128, GK * C)
        in_maps.append({"xT0": xg0, "sel": sel, "w8": w})
    return in_maps


def _assemble(outTs):
    outT = np.concatenate(outTs, axis=0)             # [D, B]
    return np.ascontiguousarray(outT.T.astype(np.float32))


def _run(x, base, mask, bitdelta, trace=False):
    nc = _get_nc(np.asarray(bitdelta, dtype=np.float32))
    in_maps = _make_in_maps(x, base, mask, bitdelta)
    res = run_bass_kernel_spmd(
        nc, in_maps, core_ids=list(range(NCORES)), trace=trace
    )
    y = _assemble([res.results[c]["outT"] for c in range(NCORES)])
    return y, res


def kernel(x, base, mask, bitdelta):
    y, _ = _run(x, base, mask, bitdelta)
    return y


# revision 17
# speedup vs baseline: 17.8866x; 17.8866x over previous
"""Trainium2 Bass kernel for a 4-layer compressed model:

    for l in range(4):  x = x @ (base[l] + bitdelta[l] * mask[l])

x: [16, 4096] f32, base/mask: [4, 4096, 4096] f32, bitdelta: [4] f32.

Sharding (8 cores, tensor parallel on weight columns):
  core c owns columns [c*512, (c+1)*512) of every layer's weight.

Key ideas vs the previous version (160 us):
  * NO on-chip weight reconstruction. The dense-combine DVE pass
    (bd*mask + base, 70 us of serial vector work at 1x mode) is gone:
      x @ (base + bd*mask) = (bd*x) @ mask  +  x @ (64*base) / 64
    Both weight streams ride fp8e4 (mask is +/-1, EXACT in fp8;
    64*base spans +-7 with ~2.4% relative error on a term that is only
    ~2.5% of the output). HBM traffic per core: 24 MiB -> 16 MiB.
  * The two matmul streams run CONCURRENTLY on the PE array via column
    tiling: bd*x stationary in array cols 0-15 (tile_position (0,0)),
    x stationary in cols 32-47 ((0,32)). Both accumulate in the same
    PSUM bank at partitions 0-15 / 32-47. Mixed dtype (bf16 stationary,
    fp8 moving) works on HW. ~32 pair-issues of N=512 per layer.
  * The mask-acc + base-acc/64 recombine AND the [16,512] -> [512,16]
    transpose for the next layer's lhsT happen in one PE pass: per
    128-column chunk, two accumulating matmuls (stack chunk stationary,
    selector moving: cols 0-15 I for the mask stack, cols 16-31 I/64
    for the base stack) produce yT = acc_mask.T + acc_base.T/64 in
    PSUM. Both stacks sit at partitions 0-15 (the base acc is drained
    PSUM[32:48] -> SBUF[0:16] by the ACT engine, which can shift
    partitions); keeping every sel-matmul in PE row group 0 matters --
    alternating stationary row groups 0/32 inside the accumulation
    stream is a hard device crash (NRT_EXEC_UNIT_UNRECOVERABLE).
  * Weights stream as 16 x 1 MiB DMAs issued up front via SWDGE
    (gpsimd): HWDGE-issued DMAs serialize globally behind in-flight
    HWDGE streams, so a big HWDGE weight stream would delay the tiny
    latency-critical staging/reload DMAs by tens of us (this was the
    baseline's hidden stall). Staging + reloads also ride the gpsimd
    queue (in program order around the collectives); only the initial
    x/sel loads and the final store use the (empty) HWDGE rings.
    3 AllGathers of yT [512,16] bf16 between layers.
  * Short PE warmer chains (anchored on each layer's yT, running on
    the next layer's already-resident weights) hold the HAM clock gate
    at 2.4 GHz through each gather window.

Output is written transposed ([512,16] f32 per core); the host
concatenates and transposes back.
"""

import numpy as np

import concourse.bass as bass
import concourse.mybir as mybir
import concourse.tile as tile
from concourse import bacc
from concourse.bass_utils import run_bass_kernel_spmd

L = 4
D = 4096
B = 16
NCORES = 8
C = D // NCORES          # 512 columns per core
KT = D // 128            # 32 contraction tiles of 128
GK = 8                   # k-tiles per DMA chunk
NCH = KT // GK           # 4 chunks per layer (1 MiB each: mask+base)
CT = C // 128            # 4 transpose chunks
SCALE = 64.0             # base pre-scale (fp8 denormal avoidance)
NWARM = 40               # PE-warmer matmuls per gather window

F32 = mybir.dt.float32
BF16 = mybir.dt.bfloat16
FP8 = mybir.dt.float8e4
ALU = mybir.AluOpType

_cache = {}


def build(bd_vals):
    nc = bacc.Bacc(
        "TRN2",
        target_bir_lowering=False,
        debug=False,
        num_devices=NCORES,
    )

    # x^T in natural [4096, 16] order; row d = p*KT + k maps to SBUF
    # partition p, matmul k-tile k -- the load is partition-contiguous.
    xT0 = nc.dram_tensor("xT0", [D, B], BF16, kind="ExternalInput")
    # selector for the recombining transposes: cols 0-15 I, cols 16-31 I/64
    sel = nc.dram_tensor("sel", [B, 2 * B], BF16, kind="ExternalInput")
    # weight chunks: [l, g, p, :GK*C] = mask[l, p*KT+g*GK+j, c] (fp8, +-1)
    #               [l, g, p, GK*C:] = 64*base[l, p*KT+g*GK+j, c] (fp8)
    w8 = nc.dram_tensor("w8", [L, NCH, 128, 2 * GK * C], FP8,
                        kind="ExternalInput")
    outT = nc.dram_tensor("outT", [C, B], F32, kind="ExternalOutput")

    rg = [list(range(NCORES))]

    with tile.TileContext(nc) as tc:
        with (
            tc.tile_pool(name="w", bufs=L * NCH) as wpool,
            tc.tile_pool(name="xp", bufs=2) as xpool,
            tc.tile_pool(name="sp", bufs=2) as spool,
            tc.tile_pool(name="const", bufs=1) as cpool,
            tc.tile_pool(name="acc", bufs=2, space="PSUM") as psum,
            tc.tile_pool(name="tp", bufs=2, space="PSUM") as tpsum,
            tc.tile_pool(name="warm", bufs=1, space="PSUM") as wpsum,
            tc.tile_pool(name="dram", bufs=2, space="DRAM") as dram,
        ):
            sel_sb = cpool.tile([B, 2 * B], BF16, tag="sel")
            nc.scalar.dma_start(sel_sb[:, :], sel[:, :])

            xt = xpool.tile([128, KT * B], BF16, tag="xt")
            nc.scalar.dma_start(
                xt[:, :].rearrange("p (k b) -> p k b", k=KT),
                xT0[:, :].rearrange("(p k) b -> p k b", p=128),
            )

            # dummy warm-up AllGather at t~0: absorbs the collective
            # subsystem's first-use initialization off the critical path
            dummy_in = dram.tile([C, B], BF16, tag="dummy_in")
            dummy_out = dram.tile([D, B], BF16, tag="dummy_out",
                                  addr_space="Shared")
            nc.scalar.dma_start(dummy_in[:, :], xT0[0:C, :])
            nc.gpsimd.collective_compute(
                "AllGather",
                ALU.bypass,
                replica_groups=rg,
                ins=[dummy_in.opt()],
                outs=[dummy_out.opt()],
            )

            # weight stream: L0+L1 chunks up front; L2/L3 chunks gated
            # behind gather-0/1 reloads so the SDMA engines have slack
            # when each AllGather's own transfers run (the CC data path
            # is starved by a saturated weight stream).
            gates = [nc.alloc_semaphore(f"wgate{i}") for i in range(2)]
            wt = []
            for l in range(L):
                for g in range(NCH):
                    t = wpool.tile([128, 2 * GK * C], FP8, tag="w")
                    d = nc.sync.dma_start(t[:, :], w8[l, g])
                    if l >= 2:
                        d._wait_ge(gates[l - 2], 16)
                    wt.append(t)

            for l in range(L):
                last = l == L - 1
                bd = float(bd_vals[l])

                xm = xpool.tile([128, KT * B], BF16, tag="xm")
                nc.vector.tensor_scalar_mul(xm[:, :], xt[:, :], bd)
                if l in (1, 2):
                    nc.vector.nop().then_inc(gates[l - 1], 16)

                ps = psum.tile([48, C], F32, tag="ps")
                for k in range(KT):
                    g, j = k // GK, k % GK
                    wti = wt[l * NCH + g]
                    nc.tensor.matmul(
                        ps[0:16, :],
                        xm[:, k * B:(k + 1) * B],
                        wti[:, j * C:(j + 1) * C],
                        start=(k == 0), stop=(k == KT - 1),
                        tile_position=(0, 0), skip_group_check=True,
                    )
                    nc.tensor.matmul(
                        ps[32:48, :],
                        xt[:, k * B:(k + 1) * B],
                        wti[:, GK * C + j * C:GK * C + (j + 1) * C],
                        start=(k == 0), stop=(k == KT - 1),
                        tile_position=(0, 32), skip_group_check=True,
                    )

                # drain both accumulators to partition-0 stacks (ACT can
                # shift partitions: PSUM[32:48] -> SBUF[0:16])
                stka = spool.tile([B, C], BF16, tag="stka")
                stkb = spool.tile([B, C], BF16, tag="stkb")
                nc.scalar.copy(stka[:, :], ps[0:16, :])
                nc.scalar.copy(stkb[:, :], ps[32:48, :])

                # recombine + transpose in one PE pass (regular matmuls,
                # stack chunk stationary, selector moving, all row grp 0):
                # yT_chunk = mask_acc.T @ I + base_acc.T @ (I/64)
                yt_ps = tpsum.tile([128, CT * B], F32, tag="ytps")
                for cc in range(CT):
                    nc.tensor.matmul(
                        yt_ps[:, cc * B:(cc + 1) * B],
                        stka[:, cc * 128:(cc + 1) * 128],
                        sel_sb[:, 0:B],
                        start=True, stop=False,
                        skip_group_check=True,
                    )
                    nc.tensor.matmul(
                        yt_ps[:, cc * B:(cc + 1) * B],
                        stkb[:, cc * 128:(cc + 1) * 128],
                        sel_sb[:, B:2 * B],
                        start=False, stop=True,
                        skip_group_check=True,
                    )

                if last:
                    ytf = spool.tile([128, CT * B], F32, tag="ytf")
                    nc.scalar.copy(ytf[:, :], yt_ps[:, :])
                    nc.scalar.dma_start(
                        outT[:, :].rearrange("(cc p) b -> p cc b", p=128),
                        ytf[:, :].rearrange("p (cc b) -> p cc b", cc=CT),
                    )
                else:
                    yt_sb = spool.tile([128, CT * B], BF16, tag="ytsb")
                    nc.scalar.copy(yt_sb[:, :], yt_ps[:, :])

                    ytb = dram.tile([C, B], BF16, tag="ytb")
                    nc.gpsimd.dma_start(
                        ytb[:, :].rearrange("(cc p) b -> p cc b", p=128),
                        yt_sb[:, :].rearrange("p (cc b) -> p cc b", cc=CT),
                    )
                    xt_full = dram.tile([D, B], BF16, tag="xtf",
                                        addr_space="Shared")
                    nc.gpsimd.collective_compute(
                        "AllGather",
                        ALU.bypass,
                        replica_groups=rg,
                        ins=[ytb.opt()],
                        outs=[xt_full.opt()],
                    )

                    # PE warmers through the gather window: anchored on
                    # yt_sb, next layer's (resident) weights as moving.
                    warm = wpsum.tile([B, 128], F32, tag="warm")
                    wsrc = wt[(l + 1) * NCH]
                    for i in range(NWARM):
                        nc.tensor.matmul(
                            warm[:, :],
                            yt_sb[:, :B],
                            wsrc[:, :128],
                            start=(i == 0), stop=(i == NWARM - 1),
                            skip_group_check=True,
                        )

                    xt = xpool.tile([128, KT * B], BF16, tag="xt")
                    nc.gpsimd.dma_start(
                        xt[:, :].rearrange("p (k b) -> p k b", k=KT),
                        xt_full[:, :].rearrange("(p k) b -> p k b", p=128),
                    )

    nc.compile()
    return nc


def _get_nc(bd_vals):
    key = tuple(float(v) for v in bd_vals)
    if _cache.get("key") != key:
        _cache["nc"] = build(bd_vals)
        _cache["key"] = key
    return _cache["nc"]


def _make_in_maps(x, base, mask, bitdelta):
    import ml_dtypes

    x = np.ascontiguousarray(np.asarray(x, dtype=np.float32))
    base = np.asarray(base, dtype=np.float32)
    mask = np.asarray(mask, dtype=np.float32)

    xT = np.ascontiguousarray(x.T).astype(ml_dtypes.bfloat16)    # [D, B]

    sel = np.zeros((B, 2 * B), dtype=np.float32)
    sel[:, 0:B] = np.eye(B, dtype=np.float32)
    sel[:, B:2 * B] = np.eye(B, dtype=np.float32) / SCALE
    sel = sel.astype(ml_dtypes.bfloat16)

    mask8 = mask.astype(ml_dtypes.float8_e4m3)
    base8 = (base * SCALE).astype(ml_dtypes.float8_e4m3)

    in_maps = []
    for c in range(NCORES):
        sl = slice(c * C, (c + 1) * C)
        m = mask8[:, :, sl].reshape(L, 128, KT, C)   # row d = p*KT + k
        b = base8[:, :, sl].reshape(L, 128, KT, C)
        w = np.empty((L, NCH, 128, 2 * GK * C), dtype=ml_dtypes.float8_e4m3)
        for g in range(NCH):
            gs = slice(g * GK, (g + 1) * GK)
            w[:, g, :, :GK * C] = m[:, :, gs, :].reshape(L, 128, GK * C)
            w[:, g, :, GK * C:] = b[:, :, gs, :].reshape(L, 128, GK * C)
        in_maps.append({"xT0": xT, "sel": sel, "w8": w})
    return in_maps


def _assemble(outTs):
    outT = np.concatenate(outTs, axis=0)             # [D, B]
    return np.ascontiguousarray(outT.T.astype(np.float32))


def _run(x, base, mask, bitdelta, trace=False):
    nc = _get_nc(np.asarray(bitdelta, dtype=np.float32))
    in_maps = _make_in_maps(x, base, mask, bitdelta)
    res = run_bass_kernel_spmd(
        nc, in_maps, core_ids=list(range(NCORES)), trace=trace
    )
    y = _assemble([res.results[c]["outT"] for c in range(NCORES)])
    return y, res


def kernel(x, base, mask, bitdelta):
    y, _ = _run(x, base, mask, bitdelta)
    return y


# revision 18
# speedup vs baseline: 18.7975x; 1.0509x over previous
"""Trainium2 Bass kernel for a 4-layer compressed model:

    for l in range(4):  x = x @ (base[l] + bitdelta[l] * mask[l])

x: [16, 4096] f32, base/mask: [4, 4096, 4096] f32, bitdelta: [4] f32.

Sharding (8 cores, tensor parallel on weight columns):
  core c owns columns [c*512, (c+1)*512) of every layer's weight.

Key ideas vs the previous version (160 us):
  * NO on-chip weight reconstruction. The dense-combine DVE pass
    (bd*mask + base, 70 us of serial vector work at 1x mode) is gone:
      x @ (base + bd*mask) = (bd*x) @ mask  +  x @ (64*base) / 64
    Both weight streams ride fp8e4 (mask is +/-1, EXACT in fp8;
    64*base spans +-7 with ~2.4% relative error on a term that is only
    ~2.5% of the output). HBM traffic per core: 24 MiB -> 16 MiB.
  * The two matmul streams run CONCURRENTLY on the PE array via column
    tiling: bd*x stationary in array cols 0-15 (tile_position (0,0)),
    x stationary in cols 32-47 ((0,32)). Both accumulate in the same
    PSUM bank at partitions 0-15 / 32-47. Mixed dtype (bf16 stationary,
    fp8 moving) works on HW. ~32 pair-issues of N=512 per layer.
  * The mask-acc + base-acc/64 recombine AND the [16,512] -> [512,16]
    transpose for the next layer's lhsT happen in one PE pass: per
    128-column chunk, two accumulating matmuls (stack chunk stationary,
    selector moving: cols 0-15 I for the mask stack, cols 16-31 I/64
    for the base stack) produce yT = acc_mask.T + acc_base.T/64 in
    PSUM. Both stacks sit at partitions 0-15 (the base acc is drained
    PSUM[32:48] -> SBUF[0:16] by the ACT engine, which can shift
    partitions); keeping every sel-matmul in PE row group 0 matters --
    alternating stationary row groups 0/32 inside the accumulation
    stream is a hard device crash (NRT_EXEC_UNIT_UNRECOVERABLE).
  * Weights stream as 16 x 1 MiB DMAs issued up front via SWDGE
    (gpsimd): HWDGE-issued DMAs serialize globally behind in-flight
    HWDGE streams, so a big HWDGE weight stream would delay the tiny
    latency-critical staging/reload DMAs by tens of us (this was the
    baseline's hidden stall). Staging + reloads also ride the gpsimd
    queue (in program order around the collectives); only the initial
    x/sel loads and the final store use the (empty) HWDGE rings.
    3 AllGathers of yT [512,16] bf16 between layers.
  * Short PE warmer chains (anchored on each layer's yT, running on
    the next layer's already-resident weights) hold the HAM clock gate
    at 2.4 GHz through each gather window.

Output is written transposed ([512,16] f32 per core); the host
concatenates and transposes back.
"""

import numpy as np

import concourse.bass as bass
import concourse.mybir as mybir
import concourse.tile as tile
from concourse import bacc
from concourse.bass_utils import run_bass_kernel_spmd

L = 4
D = 4096
B = 16
NCORES = 8
C = D // NCORES          # 512 columns per core
KT = D // 128            # 32 contraction tiles of 128
GK = 8                   # k-tiles per DMA chunk
NCH = KT // GK           # 4 chunks per layer (1 MiB each: mask+base)
CT = C // 128            # 4 transpose chunks
SCALE = 64.0             # base pre-scale (fp8 denormal avoidance)
NWARM = 40               # PE-warmer matmuls per gather window

F32 = mybir.dt.float32
BF16 = mybir.dt.bfloat16
FP8 = mybir.dt.float8e4
ALU = mybir.AluOpType

_cache = {}


def build(bd_vals):
    nc = bacc.Bacc(
        "TRN2",
        target_bir_lowering=False,
        debug=False,
        num_devices=NCORES,
    )

    # x^T in natural [4096, 16] order; row d = p*KT + k maps to SBUF
    # partition p, matmul k-tile k -- the load is partition-contiguous.
    xT0 = nc.dram_tensor("xT0", [D, B], BF16, kind="ExternalInput")
    # selector for the recombining transposes: cols 0-15 I, cols 16-31 I/64
    sel = nc.dram_tensor("sel", [B, 2 * B], BF16, kind="ExternalInput")
    # weight chunks: [l, g, p, :GK*C] = mask[l, p*KT+g*GK+j, c] (fp8, +-1)
    #               [l, g, p, GK*C:] = 64*base[l, p*KT+g*GK+j, c] (fp8)
    w8 = nc.dram_tensor("w8", [L, NCH, 128, 2 * GK * C], FP8,
                        kind="ExternalInput")
    outT = nc.dram_tensor("outT", [C, B], F32, kind="ExternalOutput")

    rg = [list(range(NCORES))]

    with tile.TileContext(nc) as tc:
        with (
            tc.tile_pool(name="w", bufs=L * NCH) as wpool,
            tc.tile_pool(name="xp", bufs=2) as xpool,
            tc.tile_pool(name="sp", bufs=2) as spool,
            tc.tile_pool(name="const", bufs=1) as cpool,
            tc.tile_pool(name="acc", bufs=2, space="PSUM") as psum,
            tc.tile_pool(name="tp", bufs=2, space="PSUM") as tpsum,
            tc.tile_pool(name="warm", bufs=1, space="PSUM") as wpsum,
            tc.tile_pool(name="dram", bufs=2, space="DRAM") as dram,
        ):
            sel_sb = cpool.tile([B, 2 * B], BF16, tag="sel")
            nc.scalar.dma_start(sel_sb[:, :], sel[:, :])

            xt = xpool.tile([128, KT * B], BF16, tag="xt")
            nc.scalar.dma_start(
                xt[:, :].rearrange("p (k b) -> p k b", k=KT),
                xT0[:, :].rearrange("(p k) b -> p k b", p=128),
            )

            # weight stream: L0+L1 chunks stream up front on the sync
            # HWDGE ring; L2/L3 chunks are issued on the gpsimd queue
            # right after the gather-0/1 reloads, so the SDMA engines
            # are idle while each AllGather's own transfers run (a
            # saturated weight stream starves the CC data path by
            # 10-20us per collective).
            wt = []
            for l in range(L):
                for g in range(NCH):
                    t = wpool.tile([128, 2 * GK * C], FP8, tag="w")
                    if l < 2:
                        nc.sync.dma_start(t[:, :], w8[l, g])
                    wt.append(t)

            for l in range(L):
                last = l == L - 1
                bd = float(bd_vals[l])

                xm = xpool.tile([128, KT * B], BF16, tag="xm")
                nc.vector.tensor_scalar_mul(xm[:, :], xt[:, :], bd)

                ps = psum.tile([48, C], F32, tag="ps")
                for k in range(KT):
                    g, j = k // GK, k % GK
                    wti = wt[l * NCH + g]
                    nc.tensor.matmul(
                        ps[0:16, :],
                        xm[:, k * B:(k + 1) * B],
                        wti[:, j * C:(j + 1) * C],
                        start=(k == 0), stop=(k == KT - 1),
                        tile_position=(0, 0), skip_group_check=True,
                    )
                    nc.tensor.matmul(
                        ps[32:48, :],
                        xt[:, k * B:(k + 1) * B],
                        wti[:, GK * C + j * C:GK * C + (j + 1) * C],
                        start=(k == 0), stop=(k == KT - 1),
                        tile_position=(0, 32), skip_group_check=True,
                    )

                # drain both accumulators to partition-0 stacks (ACT can
                # shift partitions: PSUM[32:48] -> SBUF[0:16])
                stka = spool.tile([B, C], BF16, tag="stka")
                stkb = spool.tile([B, C], BF16, tag="stkb")
                nc.scalar.copy(stka[:, :], ps[0:16, :])
                nc.scalar.copy(stkb[:, :], ps[32:48, :])

                # recombine + transpose in one PE pass (regular matmuls,
                # stack chunk stationary, selector moving, all row grp 0):
                # yT_chunk = mask_acc.T @ I + base_acc.T @ (I/64)
                yt_ps = tpsum.tile([128, CT * B], F32, tag="ytps")
                for cc in range(CT):
                    nc.tensor.matmul(
                        yt_ps[:, cc * B:(cc + 1) * B],
                        stka[:, cc * 128:(cc + 1) * 128],
                        sel_sb[:, 0:B],
                        start=True, stop=False,
                        skip_group_check=True,
                    )
                    nc.tensor.matmul(
                        yt_ps[:, cc * B:(cc + 1) * B],
                        stkb[:, cc * 128:(cc + 1) * 128],
                        sel_sb[:, B:2 * B],
                        start=False, stop=True,
                        skip_group_check=True,
                    )

                if last:
                    ytf = spool.tile([128, CT * B], F32, tag="ytf")
                    nc.scalar.copy(ytf[:, :], yt_ps[:, :])
                    nc.scalar.dma_start(
                        outT[:, :].rearrange("(cc p) b -> p cc b", p=128),
                        ytf[:, :].rearrange("p (cc b) -> p cc b", cc=CT),
                    )
                else:
                    yt_sb = spool.tile([128, CT * B], BF16, tag="ytsb")
                    nc.scalar.copy(yt_sb[:, :], yt_ps[:, :])

                    ytb = dram.tile([C, B], BF16, tag="ytb")
                    nc.gpsimd.dma_start(
                        ytb[:, :].rearrange("(cc p) b -> p cc b", p=128),
                        yt_sb[:, :].rearrange("p (cc b) -> p cc b", cc=CT),
                    )
                    xt_full = dram.tile([D, B], BF16, tag="xtf",
                                        addr_space="Shared")
                    nc.gpsimd.collective_compute(
                        "AllGather",
                        ALU.bypass,
                        replica_groups=rg,
                        ins=[ytb.opt()],
                        outs=[xt_full.opt()],
                    )

                    # PE warmers through the gather window: anchored on
                    # yt_sb, next layer's (resident) weights as moving.
                    warm = wpsum.tile([B, 128], F32, tag="warm")
                    wsrc = wt[(l + 1) * NCH]
                    for i in range(NWARM):
                        nc.tensor.matmul(
                            warm[:, :],
                            yt_sb[:, :B],
                            wsrc[:, :128],
                            start=(i == 0), stop=(i == NWARM - 1),
                            skip_group_check=True,
                        )

                    xt = xpool.tile([128, KT * B], BF16, tag="xt")
                    nc.gpsimd.dma_start(
                        xt[:, :].rearrange("p (k b) -> p k b", k=KT),
                        xt_full[:, :].rearrange("(p k) b -> p k b", p=128),
                    )
                    if l + 2 < L:
                        for g in range(NCH):
                            nc.gpsimd.dma_start(
                                wt[(l + 2) * NCH + g][:, :], w8[l + 2, g])

    nc.compile()
    return nc


def _get_nc(bd_vals):
    key = tuple(float(v) for v in bd_vals)
    if _cache.get("key") != key:
        _cache["nc"] = build(bd_vals)
        _cache["key"] = key
    return _cache["nc"]


def _make_in_maps(x, base, mask, bitdelta):
    import ml_dtypes

    x = np.ascontiguousarray(np.asarray(x, dtype=np.float32))
    base = np.asarray(base, dtype=np.float32)
    mask = np.asarray(mask, dtype=np.float32)

    xT = np.ascontiguousarray(x.T).astype(ml_dtypes.bfloat16)    # [D, B]

    sel = np.zeros((B, 2 * B), dtype=np.float32)
    sel[:, 0:B] = np.eye(B, dtype=np.float32)
    sel[:, B:2 * B] = np.eye(B, dtype=np.float32) / SCALE
    sel = sel.astype(ml_dtypes.bfloat16)

    mask8 = mask.astype(ml_dtypes.float8_e4m3)
    base8 = (base * SCALE).astype(ml_dtypes.float8_e4m3)

    in_maps = []
    for c in range(NCORES):
        sl = slice(c * C, (c + 1) * C)
        m = mask8[:, :, sl].reshape(L, 128, KT, C)   # row d = p*KT + k
        b = base8[:, :, sl].reshape(L, 128, KT, C)
        w = np.empty((L, NCH, 128, 2 * GK * C), dtype=ml_dtypes.float8_e4m3)
        for g in range(NCH):
            gs = slice(g * GK, (g + 1) * GK)
            w[:, g, :, :GK * C] = m[:, :, gs, :].reshape(L, 128, GK * C)
            w[:, g, :, GK * C:] = b[:, :, gs, :].reshape(L, 128, GK * C)
        in_maps.append({"xT0": xT, "sel": sel, "w8": w})
    return in_maps


def _assemble(outTs):
    outT = np.concatenate(outTs, axis=0)             # [D, B]
    return np.ascontiguousarray(outT.T.astype(np.float32))


def _run(x, base, mask, bitdelta, trace=False):
    nc = _get_nc(np.asarray(bitdelta, dtype=np.float32))
    in_maps = _make_in_maps(x, base, mask, bitdelta)
    res = run_bass_kernel_spmd(
        nc, in_maps, core_ids=list(range(NCORES)), trace=trace
    )
    y = _assemble([res.results[c]["outT"] for c in range(NCORES)])
    return y, res


def kernel(x, base, mask, bitdelta):
    y, _ = _run(x, base, mask, bitdelta)
    return y
